# revision 6
# baseline (speedup 1.0000x reference)
"""Single-head attention with LoRA-folded projections on 8 TRN2 NeuronCores.

Problem: nn_Attention_Head (B=8, S=2048, EMB=1024, HEAD=64, RANK=8).
Sharding: data-parallel over batch — core b computes batch element b.

Math (per batch):
  Weff_x = Wx + 2.0 * (Bx @ Ax)            (LoRA folded on host — exact algebra)
  q = x @ Weff_q^T ; k = x @ Weff_k^T ; v = x @ Weff_v^T
  S = q @ k^T / 8, masked where tokMrk==0, softmax over keys, out = S @ v

Device pipeline (per core):
  1. x [2048,1024] loaded naturally, PE-transposed into xT [emb,tok] (fp32r).
  2. qT/kT/vT [64,2048] = Weff^T_chunk^T @ xT_chunk (fp32r matmuls, M=64).
  3. v transposed back to v_nat [tok,64] with a ones-column appended ->
     the PV matmul then also produces the softmax denominators.
  4. Attention in transposed layout: S^T[k,q] = kT^T @ qT; P^T = exp(S^T/8 + mask_bias)
     (mask as per-partition activation bias, -60 for masked keys -> exp ~ 0);
     outT[65,q] += (v|1)^T @ P^T accumulated over k-tiles.
  5. outT PE-transposed to [q,65]; out = outT[:, :64] * (1/outT[:, 64]); DMA out.
"""

import numpy as np
from contextlib import ExitStack

import concourse.bass as bass
import concourse.mybir as mybir
import concourse.tile as tile
from concourse import bacc, bass_utils
from concourse.masks import make_identity

B, S, EMB, HEAD = 8, 2048, 1024, 64
LORA_SCALE = 2.0
MASK_BIAS = -60.0
N_CORES = 8
KT = S // 128          # 16 k-tiles of 128 tokens
QB = S // 512          # 4 q-blocks of 512 tokens
NCH = EMB // 128       # 8 emb chunks

F32 = mybir.dt.float32
F32R = mybir.dt.float32r
EXP = mybir.ActivationFunctionType.Exp

# test.py can override these to enable tracing
RUN_KWARGS = {}


def _copy(nc, use_vector, dst, src):
    if use_vector:
        nc.vector.tensor_copy(dst, src)
    else:
        nc.scalar.copy(dst, src)


def build_nc():
    nc = bacc.Bacc("TRN2", target_bir_lowering=False, debug=False)

    x_d = nc.dram_tensor("x", [S, EMB], F32, kind="ExternalInput").ap()
    wt_d = nc.dram_tensor("wt", [128, NCH, 3 * HEAD], F32R, kind="ExternalInput").ap()
    mask_d = nc.dram_tensor("mask", [128, KT], F32, kind="ExternalInput").ap()
    ones_d = nc.dram_tensor("ones", [128, KT, 1], F32R, kind="ExternalInput").ap()
    out_d = nc.dram_tensor("out", [S, HEAD], F32, kind="ExternalOutput").ap()

    with tile.TileContext(nc) as tc, ExitStack() as ctx:
        consts = ctx.enter_context(tc.tile_pool(name="consts", bufs=1))
        xin = ctx.enter_context(tc.tile_pool(name="xin", bufs=3))
        xtp = ctx.enter_context(tc.tile_pool(name="xt", bufs=1))
        qkv = ctx.enter_context(tc.tile_pool(name="qkv", bufs=1))
        ptp = ctx.enter_context(tc.tile_pool(name="pt", bufs=3))
        osum = ctx.enter_context(tc.tile_pool(name="osum", bufs=2))
        oout = ctx.enter_context(tc.tile_pool(name="oout", bufs=4))

        ps_x = ctx.enter_context(tc.tile_pool(name="ps_x", bufs=2, space="PSUM"))
        ps_p = ctx.enter_context(tc.tile_pool(name="ps_p", bufs=2, space="PSUM"))
        ps_st = ctx.enter_context(tc.tile_pool(name="ps_st", bufs=2, space="PSUM"))
        ps_o = ctx.enter_context(tc.tile_pool(name="ps_o", bufs=2, space="PSUM"))

        ident = consts.tile([128, 128], F32)
        make_identity(nc, ident[:])
        wt_sb = consts.tile([128, NCH, 3 * HEAD], F32R)
        nc.sync.dma_start(out=wt_sb[:], in_=wt_d)
        mask_sb = consts.tile([128, KT], F32)
        nc.sync.dma_start(out=mask_sb[:], in_=mask_d)

        # ---- Phase 1: load x, transpose into xT [128, NCH, S] (fp32r) ----
        xt_sb = xtp.tile([128, NCH, S], F32R)
        for t in range(KT):
            x_t = xin.tile([128, EMB], F32)
            nc.sync.dma_start(out=x_t[:], in_=x_d[t * 128:(t + 1) * 128, :])
            for g in range(NCH // 4):
                px = ps_x.tile([128, 512], F32, tag="tp")
                for cc in range(4):
                    c = g * 4 + cc
                    nc.tensor.matmul(
                        out=px[:, cc * 128:(cc + 1) * 128],
                        lhsT=x_t[:, c * 128:(c + 1) * 128],
                        rhs=ident[:],
                        is_transpose=True,
                        start=(cc == 0), stop=(cc == 3),
                    )
                src = px[:].rearrange("p (c f) -> p c f", c=4)
                dst = xt_sb[:, g * 4:(g + 1) * 4, t * 128:(t + 1) * 128]
                _copy(nc, (t * 2 + g) % 2 == 0, dst, src)

        # ---- Phase 2: projections qT/kT/vT [64, S] ----
        qT = qkv.tile([64, S], F32R)
        kT = qkv.tile([64, S], F32R)
        vT = qkv.tile([64, S], F32)
        for wi, dest in enumerate((qT, kT, vT)):
            for nb in range(QB):
                pp = ps_p.tile([64, 512], F32, tag="pp")
                for c in range(NCH):
                    nc.tensor.matmul(
                        out=pp[:],
                        lhsT=wt_sb[:, c, wi * HEAD:(wi + 1) * HEAD],
                        rhs=xt_sb[:, c, nb * 512:(nb + 1) * 512],
                        start=(c == 0), stop=(c == NCH - 1),
                    )
                _copy(nc, (wi * QB + nb) % 2 == 0,
                      dest[:, nb * 512:(nb + 1) * 512], pp[:])

        # ---- Phase 2b: v_nat [128, KT, 65] with ones column ----
        v1 = qkv.tile([128, KT, HEAD + 1], F32R)
        # ones column via DMA (memset can't produce float32r)
        nc.sync.dma_start(out=v1[:, :, HEAD:HEAD + 1], in_=ones_d)
        for g in range(KT // 4):
            pv = ps_x.tile([128, 512], F32, tag="tp")
            for j in range(4):
                kt = g * 4 + j
                nc.tensor.matmul(
                    out=pv[:, j * HEAD:(j + 1) * HEAD],
                    lhsT=vT[:, kt * 128:(kt + 1) * 128],
                    rhs=ident[0:64, 0:64],
                    is_transpose=True,
                    start=(j == 0), stop=(j == 3),
                )
            src = pv[:, 0:4 * HEAD].rearrange("p (j f) -> p j f", j=4)
            _copy(nc, g % 2 == 0, v1[:, g * 4:(g + 1) * 4, 0:HEAD], src)

        # ---- Phase 3: attention per q-block ----
        for qb in range(QB):
            po = ps_o.tile([HEAD + 1, 512], F32, tag="po")
            for kt in range(KT):
                pst = ps_st.tile([128, 512], F32, tag="pst")
                nc.tensor.matmul(
                    out=pst[:],
                    lhsT=kT[:, kt * 128:(kt + 1) * 128],
                    rhs=qT[:, qb * 512:(qb + 1) * 512],
                    start=True, stop=True,
                )
                ptile = ptp.tile([128, 512], F32R)
                nc.scalar.activation(
                    out=ptile[:], in_=pst[:], func=EXP,
                    bias=mask_sb[:, kt:kt + 1], scale=1.0 / np.sqrt(HEAD),
                )
                nc.tensor.matmul(
                    out=po[:],
                    lhsT=v1[:, kt, :],
                    rhs=ptile[:],
                    start=(kt == 0), stop=(kt == KT - 1),
                )
            os_sb = osum.tile([HEAD + 1, 512], F32)
            nc.vector.tensor_copy(os_sb[:], po[:])
            for j in range(4):
                pt2 = ps_x.tile([128, 512], F32, tag="tp")
                nc.tensor.matmul(
                    out=pt2[:, 0:HEAD + 1],
                    lhsT=os_sb[:, j * 128:(j + 1) * 128],
                    rhs=ident[0:HEAD + 1, 0:HEAD + 1],
                    is_transpose=True,
                    start=True, stop=True,
                )
                inv = oout.tile([128, 1], F32)
                nc.vector.reciprocal(inv[:], pt2[:, HEAD:HEAD + 1])
                ob = oout.tile([128, HEAD], F32)
                nc.vector.tensor_scalar_mul(ob[:], pt2[:, 0:HEAD], inv[:])
                r0 = qb * 512 + j * 128
                nc.sync.dma_start(out=out_d[r0:r0 + 128, :], in_=ob[:])

    nc.compile()
    return nc


def prep_inputs(batEmb, tokMrk, Wq, Wk, Wv, Aq, Bq, Ak, Bk, Av, Bv):
    """Fold LoRA into the base weights, lay out per-core input maps."""
    ws = []
    for W, A, Bm in ((Wq, Aq, Bq), (Wk, Ak, Bk), (Wv, Av, Bv)):
        ws.append(W.astype(np.float64) + LORA_SCALE * (Bm.astype(np.float64) @ A.astype(np.float64)))
    wcat = np.concatenate(ws, axis=0).astype(np.float32)          # [192, 1024]
    wt = np.ascontiguousarray(
        wcat.T.reshape(NCH, 128, 3 * HEAD).transpose(1, 0, 2))    # [128, NCH, 192]

    in_maps = []
    for b in range(B):
        mask = np.where(tokMrk[b] == 0, np.float32(MASK_BIAS), np.float32(0.0))
        mask = np.ascontiguousarray(mask.reshape(KT, 128).T)      # [128, KT]
        in_maps.append({
            "x": np.ascontiguousarray(batEmb[b]),
            "wt": wt,
            "mask": mask,
            "ones": np.ones((128, KT, 1), np.float32),
        })
    return in_maps


_CACHED_NC = None


def kernel(**inputs):
    global _CACHED_NC
    if _CACHED_NC is None:
        _CACHED_NC = build_nc()
    nc = _CACHED_NC
    in_maps = prep_inputs(**{k: np.asarray(v) for k, v in inputs.items()})
    res = bass_utils.run_bass_kernel_spmd(
        nc, in_maps, core_ids=list(range(N_CORES)), **RUN_KWARGS)
    kernel.last_results = res
    return np.stack([res.results[b]["out"] for b in range(N_CORES)])


# revision 7
# speedup vs baseline: 1.7552x; 1.7552x over previous
"""Single-head attention with LoRA-folded projections on 8 TRN2 NeuronCores.

Problem: nn_Attention_Head (B=8, S=2048, EMB=1024, HEAD=64, RANK=8).
Sharding: data-parallel over batch — core b computes batch element b.

Math (per batch):
  Weff_x = Wx + 2.0 * (Bx @ Ax)            (LoRA folded on host — exact algebra)
  q = x @ Weff_q^T ; k = x @ Weff_k^T ; v = x @ Weff_v^T
  S = q @ k^T / 8, masked where tokMrk==0, softmax over keys, out = S @ v

Device pipeline (per core):
  1. x loaded in token tiles (cast to bf16 in the DMA), PE-transposed into
     xT [emb, tok] (bf16).
  2. Packed [Wq|Wk] projection (M=128) -> q rows 0-63 / k rows 64-127 in PSUM;
     q half copied to qT1 (fp32r), k half staged and realigned to partitions
     0-63 of kTb via SBUF->SBUF DMA.  v projection separately (M=64).
     qT1 row 64 = ones, kTb row 64 = mask bias (-480 for masked keys), so the
     S^T matmul adds the mask for free (K = 65).
  3. v transposed back to v_nat [tok, 64] with a ones column appended ->
     the PV matmul also produces the softmax denominators.
  4. Attention per q-block, software-pipelined S^T two k-tiles ahead of PV:
     S^T[k,q] = kTb^T @ qT1 (mask folded in); P^T = exp(S^T / 8) on ACT
     (the -480 bias scales to -60 -> exp == 0 for masked keys);
     outT[65,q] += (v|1)^T @ P^T accumulated over k-tiles.
  5. outT PE-transposed to [q,65]; out = outT[:, :64] * (1/outT[:, 64]); DMA.
"""

import numpy as np
from contextlib import ExitStack

import concourse.bass as bass
import concourse.mybir as mybir
import concourse.tile as tile
from concourse import bacc, bass_utils

B, S, EMB, HEAD = 8, 2048, 1024, 64
LORA_SCALE = 2.0
MASK_BIAS = -480.0     # pre-softmax-scale; * 0.125 -> -60 added to the logits
N_CORES = 8
KT = S // 128          # 16 k-tiles of 128 tokens
QB = S // 512          # 4 blocks of 512 tokens
NCH = EMB // 128       # 8 emb chunks

F32 = mybir.dt.float32
F32R = mybir.dt.float32r
BF16 = mybir.dt.bfloat16
EXP = mybir.ActivationFunctionType.Exp

# test.py can override these to enable tracing
RUN_KWARGS = {}


def _copy(nc, use_vector, dst, src):
    if use_vector:
        nc.vector.tensor_copy(dst, src)
    else:
        nc.scalar.copy(dst, src)


def build_nc():
    nc = bacc.Bacc("TRN2", target_bir_lowering=False, debug=False)

    x_d = nc.dram_tensor("x", [S, EMB], F32, kind="ExternalInput").ap()
    wt_d = nc.dram_tensor("wt", [128, NCH, 3 * HEAD], BF16, kind="ExternalInput").ap()
    maskrow_d = nc.dram_tensor("maskrow", [1, S], F32R, kind="ExternalInput").ap()
    onesrow_d = nc.dram_tensor("onesrow", [1, S], F32R, kind="ExternalInput").ap()
    onescol_d = nc.dram_tensor("onescol", [128, KT, 1], F32R, kind="ExternalInput").ap()
    ident_d = nc.dram_tensor("ident", [128, 128], BF16, kind="ExternalInput").ap()
    identf_d = nc.dram_tensor("identf", [128, 128], F32, kind="ExternalInput").ap()
    out_d = nc.dram_tensor("out", [S, HEAD], F32, kind="ExternalOutput").ap()

    with tile.TileContext(nc) as tc, ExitStack() as ctx:
        consts = ctx.enter_context(tc.tile_pool(name="consts", bufs=1))
        xin = ctx.enter_context(tc.tile_pool(name="xin", bufs=4))
        xtp = ctx.enter_context(tc.tile_pool(name="xt", bufs=1))
        qkv = ctx.enter_context(tc.tile_pool(name="qkv", bufs=1))
        ptp = ctx.enter_context(tc.tile_pool(name="pt", bufs=5))
        osum = ctx.enter_context(tc.tile_pool(name="osum", bufs=2))
        oout = ctx.enter_context(tc.tile_pool(name="oout", bufs=4))

        # PSUM: 3 + 3 + 2 = 8 banks
        ps_sc = ctx.enter_context(tc.tile_pool(name="ps_sc", bufs=3, space="PSUM"))
        ps_st = ctx.enter_context(tc.tile_pool(name="ps_st", bufs=3, space="PSUM"))
        ps_o = ctx.enter_context(tc.tile_pool(name="ps_o", bufs=2, space="PSUM"))

        ident = consts.tile([128, 128], BF16)
        nc.sync.dma_start(out=ident[:], in_=ident_d)
        identf = consts.tile([128, 128], F32)
        nc.sync.dma_start(out=identf[:], in_=identf_d)
        wt_sb = consts.tile([128, NCH, 3 * HEAD], BF16)
        nc.sync.dma_start(out=wt_sb[:], in_=wt_d)

        qT1 = qkv.tile([HEAD + 1, S], F32R)
        kTb = qkv.tile([HEAD + 1, S], F32R)
        ktmp = qkv.tile([128, S], F32R)      # k staged on partitions 64-127
        vT = qkv.tile([64, S], F32)
        v1 = qkv.tile([128, KT, HEAD + 1], F32R)
        nc.sync.dma_start(out=qT1[HEAD:HEAD + 1, :], in_=onesrow_d)
        nc.sync.dma_start(out=kTb[HEAD:HEAD + 1, :], in_=maskrow_d)
        nc.sync.dma_start(out=v1[:, :, HEAD:HEAD + 1], in_=onescol_d)

        # ---- Phase 1+2 fused per 512-token block: load+transpose+project ----
        xt_sb = xtp.tile([128, NCH, S], BF16)
        ci = 0
        for nb in range(QB):
            for tt in range(4):
                t = nb * 4 + tt
                x_t = xin.tile([128, EMB], BF16)
                # SWDGE cast-DMA fp32 -> bf16
                nc.gpsimd.dma_start(out=x_t[:], in_=x_d[t * 128:(t + 1) * 128, :])
                for g in range(NCH // 4):
                    px = ps_sc.tile([128, 512], BF16, tag="sc")
                    for cc in range(4):
                        c = g * 4 + cc
                        nc.tensor.matmul(
                            out=px[:, cc * 128:(cc + 1) * 128],
                            lhsT=x_t[:, c * 128:(c + 1) * 128],
                            rhs=ident[:],
                            is_transpose=True,
                            start=(cc == 0), stop=(cc == 3),
                        )
                    src = px[:].rearrange("p (c f) -> p c f", c=4)
                    dst = xt_sb[:, g * 4:(g + 1) * 4, t * 128:(t + 1) * 128]
                    _copy(nc, ci % 2 == 0, dst, src)
                    ci += 1
            # packed [q|k] projection for this token block (M=128)
            pp = ps_sc.tile([128, 512], F32, tag="sc")
            for c in range(NCH):
                nc.tensor.matmul(
                    out=pp[:],
                    lhsT=wt_sb[:, c, 0:128],
                    rhs=xt_sb[:, c, nb * 512:(nb + 1) * 512],
                    start=(c == 0), stop=(c == NCH - 1),
                )
            _copy(nc, True, qT1[0:HEAD, nb * 512:(nb + 1) * 512], pp[0:HEAD, :])
            _copy(nc, False, ktmp[HEAD:128, nb * 512:(nb + 1) * 512], pp[HEAD:128, :])
            # realign k to partitions 0-63 (SBUF->SBUF DMA moves partitions)
            nc.sync.dma_start(
                out=kTb[0:HEAD, nb * 512:(nb + 1) * 512],
                in_=ktmp[HEAD:128, nb * 512:(nb + 1) * 512],
            )
            # v projection (M=64)
            pv = ps_sc.tile([128, 512], F32, tag="sc")
            for c in range(NCH):
                nc.tensor.matmul(
                    out=pv[0:HEAD, :],
                    lhsT=wt_sb[:, c, 128:192],
                    rhs=xt_sb[:, c, nb * 512:(nb + 1) * 512],
                    start=(c == 0), stop=(c == NCH - 1),
                )
            _copy(nc, True, vT[:, nb * 512:(nb + 1) * 512], pv[0:HEAD, :])

        # ---- Phase 2b: v_nat [128, KT, 65] with ones column ----
        for g in range(KT // 4):
            pw = ps_sc.tile([128, 512], F32, tag="sc")
            for j in range(4):
                kt = g * 4 + j
                nc.tensor.matmul(
                    out=pw[:, j * HEAD:(j + 1) * HEAD],
                    lhsT=vT[:, kt * 128:(kt + 1) * 128],
                    rhs=identf[0:64, 0:64],
                    is_transpose=True,
                    start=(j == 0), stop=(j == 3),
                )
            src = pw[:, 0:4 * HEAD].rearrange("p (j f) -> p j f", j=4)
            _copy(nc, g % 2 == 0, v1[:, g * 4:(g + 1) * 4, 0:HEAD], src)

        # ---- Phase 3: attention per q-block, S^T pipelined 2 ahead of PV ----
        for qb in range(QB):
            po = ps_o.tile([HEAD + 1, 512], F32, tag="po")
            ptiles = {}

            def emit_s(kt, qb=qb, ptiles=ptiles):
                pst = ps_st.tile([128, 512], F32, tag="st")
                nc.tensor.matmul(
                    out=pst[:],
                    lhsT=kTb[:, kt * 128:(kt + 1) * 128],
                    rhs=qT1[:, qb * 512:(qb + 1) * 512],
                    start=True, stop=True,
                )
                pt_t = ptp.tile([128, 512], F32R)
                nc.scalar.activation(
                    out=pt_t[:], in_=pst[:], func=EXP, scale=1.0 / np.sqrt(HEAD))
                ptiles[kt] = pt_t

            emit_s(0)
            emit_s(1)
            for kt in range(KT):
                nc.tensor.matmul(
                    out=po[:],
                    lhsT=v1[:, kt, :],
                    rhs=ptiles.pop(kt)[:],
                    start=(kt == 0), stop=(kt == KT - 1),
                )
                if kt + 2 < KT:
                    emit_s(kt + 2)

            os_sb = osum.tile([HEAD + 1, 512], F32)
            nc.vector.tensor_copy(os_sb[:], po[:])
            for j in range(4):
                pt2 = ps_sc.tile([128, 512], F32, tag="sc")
                nc.tensor.matmul(
                    out=pt2[:, 0:HEAD + 1],
                    lhsT=os_sb[:, j * 128:(j + 1) * 128],
                    rhs=identf[0:HEAD + 1, 0:HEAD + 1],
                    is_transpose=True,
                    start=True, stop=True,
                )
                inv = oout.tile([128, 1], F32)
                nc.vector.reciprocal(inv[:], pt2[:, HEAD:HEAD + 1])
                ob = oout.tile([128, HEAD], F32)
                nc.vector.tensor_scalar_mul(ob[:], pt2[:, 0:HEAD], inv[:])
                r0 = qb * 512 + j * 128
                nc.sync.dma_start(out=out_d[r0:r0 + 128, :], in_=ob[:])

    nc.compile()
    return nc


def prep_inputs(batEmb, tokMrk, Wq, Wk, Wv, Aq, Bq, Ak, Bk, Av, Bv):
    """Fold LoRA into the base weights, lay out per-core input maps."""
    ws = []
    for W, A, Bm in ((Wq, Aq, Bq), (Wk, Ak, Bk), (Wv, Av, Bv)):
        ws.append(W.astype(np.float64) + LORA_SCALE * (Bm.astype(np.float64) @ A.astype(np.float64)))
    wcat = np.concatenate(ws, axis=0).astype(np.float32)          # [192, 1024]
    wt = np.ascontiguousarray(
        wcat.T.reshape(NCH, 128, 3 * HEAD).transpose(1, 0, 2))    # [128, NCH, 192]
    import ml_dtypes
    wt = wt.astype(ml_dtypes.bfloat16)
    ident = np.eye(128, dtype=ml_dtypes.bfloat16)
    identf = np.eye(128, dtype=np.float32)

    in_maps = []
    for b in range(B):
        maskrow = np.where(tokMrk[b] == 0, np.float32(MASK_BIAS),
                           np.float32(0.0)).reshape(1, S)
        in_maps.append({
            "x": np.ascontiguousarray(batEmb[b]),
            "wt": wt,
            "maskrow": np.ascontiguousarray(maskrow),
            "onesrow": np.ones((1, S), np.float32),
            "onescol": np.ones((128, KT, 1), np.float32),
            "ident": ident,
            "identf": identf,
        })
    return in_maps


_CACHED_NC = None


def kernel(**inputs):
    global _CACHED_NC
    if _CACHED_NC is None:
        _CACHED_NC = build_nc()
    nc = _CACHED_NC
    in_maps = prep_inputs(**{k: np.asarray(v) for k, v in inputs.items()})
    res = bass_utils.run_bass_kernel_spmd(
        nc, in_maps, core_ids=list(range(N_CORES)), **RUN_KWARGS)
    kernel.last_results = res
    return np.stack([res.results[b]["out"] for b in range(N_CORES)])
